# revision 1
# baseline (speedup 1.0000x reference)
"""BCQLinear packed forward on 8 Trainium2 NeuronCores.

Column-parallel (tensor-parallel) sharding: binary/alpha/bias are sharded
along out_features (dim 0, 4096 -> 8 x 512); the input activations are
replicated. Each core dequantizes its weight shard
    W[o, i] = sum_b alpha[o, g, b] * binary[o, g, a, b],   i = 128 g + a
on the vector engine, transposes it through the PE array to Wt[i, o], and
runs a K-contiguous fp32r matmul  out[ms, o] = x[ms, :] @ Wt[:, o] + bias.
The host concatenates the 8 output shards along o.

Shapes are hardcoded for the problem instance:
  input  [2, 1024, 4096] f32
  binary [4096, 32, 128, 3] f32 (+-1)
  alpha  [4096, 32, 3] f32
  bias   [4096] f32
"""

import numpy as np
from contextlib import ExitStack

import bass_rust
import concourse.bass as bass
import concourse.mybir as mybir
import concourse.tile as tile
from concourse.bass_utils import run_bass_kernel_spmd
from concourse.masks import make_identity


def _legalize_waits(nc, max_waits=1):
    """Walrus codegen allows only one sync-wait on (at least) DVE
    TensorTensor instructions. Move excess waits onto injected same-engine
    NoOps placed immediately before the instruction (program order per
    engine preserves the semantics)."""
    seq = 0
    for fn in nc.m.functions:
        for blk in fn.blocks:
            new_insts = []
            changed = False
            for inst in blk.instructions:
                si = inst.sync_info
                if si is not None and len(si.on_wait) > max_waits:
                    waits = list(si.on_wait)
                    for w in waits[:-max_waits]:
                        nop = mybir.InstNoOp(name=f"wlegal-{seq}")
                        seq += 1
                        nop.engine = inst.engine
                        nop.sync_info = bass_rust.SyncInfo(
                            on_wait=[w], on_update=[])
                        new_insts.append(nop)
                    inst.sync_info = bass_rust.SyncInfo(
                        on_wait=waits[-max_waits:],
                        on_update=list(si.on_update))
                    changed = True
                new_insts.append(inst)
            if changed:
                blk.instructions = new_insts

P = 128          # partitions
N_CORES = 8
B, S = 2, 1024
MS = B * S       # 2048 tokens
I = 4096         # in_features
O = 4096         # out_features
O_SH = O // N_CORES  # 512 per core
G, A, NB = 32, 128, 3
KT = I // P      # 32 contraction tiles
MB = MS // P     # 16 token blocks
OT = O_SH // P   # 4 o-tiles per core

F32 = mybir.dt.float32
F32R = mybir.dt.float32r
BIN_DTYPES = {
    "f32": mybir.dt.float32,
    "bf16": mybir.dt.bfloat16,
    "fp8": mybir.dt.float8e4,
}

_CACHED = {}


def build_nc(mm_f32r: bool = True, bin_dtype: str = "fp8",
             fuse_transpose: bool = True, x_bufs: int = 3,
             pool_ots: tuple = (), pool_planes: int = 0,
             repeat: int = 1, plane_bufs: int = 7,
             b_bufs: int = 5, plane_split: int = 2) -> bass.Bass:
    nc = bass.Bass("TRN2", target_bir_lowering=False, debug=False)
    MMDT = F32R if mm_f32r else F32
    BDT = BIN_DTYPES[bin_dtype]

    # Host-staged layouts (pure relayouts of the sharded inputs):
    #  xb    [MB, P, KT, P]  : xb[m, p, k, j] = x[m*128+j, k*128+p]
    #  bperm [O_SH, NB, G, A]: binary shard with the bit axis moved outward
    #  alpha [O_SH, G, NB]
    #  biasr [P, O_SH]       : bias shard replicated across partitions
    xb_d = nc.dram_tensor("xb", [MB, P, KT, P], MMDT, kind="ExternalInput").ap()
    b_d = nc.dram_tensor("bperm", [O_SH, NB, G, A], BDT, kind="ExternalInput").ap()
    al_d = nc.dram_tensor("alpha", [O_SH, G, NB], F32, kind="ExternalInput").ap()
    bias_d = nc.dram_tensor("biasr", [P, O_SH], F32, kind="ExternalInput").ap()
    out_d = nc.dram_tensor("out", [MS, O_SH], F32, kind="ExternalOutput").ap()
    out_t = out_d.rearrange("(mb p) o -> mb p o", p=P)

    mult = mybir.AluOpType.mult
    add = mybir.AluOpType.add

    with tile.TileContext(nc) as tc, ExitStack() as ctx:
        const = ctx.enter_context(tc.tile_pool(name="const", bufs=1))
        wt_pool = ctx.enter_context(tc.tile_pool(name="wt", bufs=1))
        bpool = ctx.enter_context(tc.tile_pool(name="bin", bufs=b_bufs))
        wpool = ctx.enter_context(
            tc.tile_pool(name="w", bufs=plane_bufs if fuse_transpose else 2))
        xpool = ctx.enter_context(tc.tile_pool(name="x", bufs=x_bufs))
        opool = ctx.enter_context(tc.tile_pool(name="o", bufs=2))
        ps_mm = ctx.enter_context(tc.tile_pool(name="psmm", bufs=2, space="PSUM"))
        ps_tr = ctx.enter_context(tc.tile_pool(name="pstr", bufs=4, space="PSUM"))

        ident = const.tile([P, P], F32)
        make_identity(nc, ident)
        if mm_f32r:
            # is_transpose matmuls on f32r planes need an f32r identity and
            # an explicitly-rounded producer (DVE copy rounds to f32r).
            ident_mm = const.tile([P, P], F32R, tag="identr")
            nc.vector.tensor_copy(ident_mm, ident)
        else:
            ident_mm = ident
        bias_f32 = const.tile([P, O_SH], F32)
        nc.sync.dma_start(bias_f32, bias_d)
        al_sb = const.tile([P, OT, G * NB], F32)
        nc.sync.dma_start(al_sb, al_d.rearrange("(ot p) g nb -> p ot (g nb)", p=P))

        # ---- Dequant + transpose, one o-tile (128 out channels) at a time
        al3 = al_sb.rearrange("p ot (g nb) -> p ot g nb", nb=NB)

        def emit_dequant(wt_sb, ot):
            def al_bc(b, ot=ot):
                # alpha[o_p, g, b] broadcast along a: [P, G, 1] -> [P, G, A]
                return al3[:, ot, :, b:b + 1].to_broadcast([P, G, A])

            eng = nc.gpsimd if ot in pool_ots else nc.vector
            if fuse_transpose:
                # Scale each +-1 bit-plane by its alpha (DVE), then let the
                # PE transpose-accumulate the three planes in PSUM:
                # Wt[a, o] = sum_b (alpha_b * B_b)[o, a].
                # Planes are emitted in half-G granularity so DVE scaling,
                # PE transposes and ACT copies pipeline across o-tiles.
                GH = G // plane_split
                b_tiles = []
                for b in range(NB):
                    b_sb = bpool.tile([P, G, A], BDT)
                    nc.sync.dma_start(b_sb, b_d[ot * P:(ot + 1) * P, b])
                    b_tiles.append(b_sb)
                for gh in range(plane_split):
                    gsl = slice(gh * GH, (gh + 1) * GH)
                    planes = []
                    for b in range(NB):
                        pl = wpool.tile([P, GH, A], MMDT, tag="plane")
                        e = nc.gpsimd if b >= NB - pool_planes else eng
                        e.tensor_tensor(pl, b_tiles[b][:, gsl],
                                        al3[:, ot, gsl, b:b + 1]
                                        .to_broadcast([P, GH, A]), mult)
                        planes.append(pl)
                    for gg in range(GH):
                        g = gh * GH + gg
                        ps = ps_tr.tile([P, P], MMDT)
                        for b in range(NB):
                            nc.tensor.matmul(ps, planes[b][:, gg], ident_mm,
                                             is_transpose=True,
                                             start=(b == 0), stop=(b == NB - 1))
                        nc.scalar.copy(wt_sb[:, g, ot * P:(ot + 1) * P], ps)
            else:
                b_tiles = []
                for b in range(NB):
                    b_sb = bpool.tile([P, G, A], BDT)
                    nc.sync.dma_start(b_sb, b_d[ot * P:(ot + 1) * P, b])
                    b_tiles.append(b_sb)
                w_sb = wpool.tile([P, G, A], F32)  # W[o_p, g, a]
                if BDT == F32:
                    eng.tensor_tensor(b_tiles[1], b_tiles[1], al_bc(1), mult)
                    eng.tensor_tensor(b_tiles[2], b_tiles[2], al_bc(2), mult)
                    eng.tensor_tensor(w_sb, b_tiles[0], al_bc(0), mult)
                    eng.tensor_tensor(w_sb, w_sb, b_tiles[1], add)
                    eng.tensor_tensor(w_sb, w_sb, b_tiles[2], add)
                else:
                    t_sb = wpool.tile([P, G, A], F32, tag="tmp")
                    eng.tensor_tensor(w_sb, b_tiles[0], al_bc(0), mult)
                    eng.tensor_tensor(t_sb, b_tiles[1], al_bc(1), mult)
                    eng.tensor_tensor(w_sb, w_sb, t_sb, add)
                    eng.tensor_tensor(t_sb, b_tiles[2], al_bc(2), mult)
                    eng.tensor_tensor(w_sb, w_sb, t_sb, add)
                for g in range(G):
                    ps = ps_tr.tile([P, P], F32)
                    nc.tensor.transpose(ps, w_sb[:, g], ident)
                    nc.scalar.copy(wt_sb[:, g, ot * P:(ot + 1) * P], ps)

        def emit_mm(wt_sb, m):
            # Matmul for one 128-token block: contract all 32 k-tiles
            xt_sb = xpool.tile([P, KT, P], MMDT)
            nc.sync.dma_start(xt_sb, xb_d[m])
            ps = ps_mm.tile([P, O_SH], F32)
            for k in range(KT):
                nc.tensor.matmul(ps, xt_sb[:, k], wt_sb[:, k],
                                 start=(k == 0), stop=(k == KT - 1))
            out_sb = opool.tile([P, O_SH], F32)
            nc.vector.tensor_tensor(out_sb, ps, bias_f32, add)
            nc.sync.dma_start(out_t[m], out_sb)

        for _rep in range(repeat):
            # Wt[i, o] resident: [P(i%128), KT, O_SH]
            wt_sb = wt_pool.tile([P, KT, O_SH], MMDT, tag="wt")
            for ot in range(OT):
                emit_dequant(wt_sb, ot)
            for m in range(MB):
                emit_mm(wt_sb, m)

    _legalize_waits(nc)
    return nc


def _stage_inputs(input, binary, alpha, bias, bin_dtype="fp8"):
    x = np.ascontiguousarray(np.asarray(input, dtype=np.float32)).reshape(MS, I)
    # xb[m, p, k, j] = x[m*128+j, k*128+p]
    xb = np.ascontiguousarray(
        x.reshape(MB, P, KT, P).transpose(0, 3, 2, 1))
    # binary is strictly +-1, exactly representable in bf16/fp8e4 — the cast
    # during staging is lossless.
    np_bdt = mybir.dt.np(BIN_DTYPES[bin_dtype])
    bperm = np.ascontiguousarray(
        np.asarray(binary, dtype=np.float32).transpose(0, 3, 1, 2)
    ).astype(np_bdt)
    alpha = np.ascontiguousarray(np.asarray(alpha, dtype=np.float32))
    bias = np.asarray(bias, dtype=np.float32)

    in_maps = []
    for c in range(N_CORES):
        sl = slice(c * O_SH, (c + 1) * O_SH)
        in_maps.append({
            "xb": xb,
            "bperm": np.ascontiguousarray(bperm[sl]),
            "alpha": np.ascontiguousarray(alpha[sl]),
            "biasr": np.ascontiguousarray(
                np.broadcast_to(bias[sl][None, :], (P, O_SH))),
        })
    return in_maps


def kernel(input, binary, alpha, bias, _trace=False, _mm_f32r=True,
           _bin_dtype="fp8", _fuse_transpose=True, _x_bufs=3, _pool_ots=(),
           _pool_planes=0):
    key = (_mm_f32r, _bin_dtype, _fuse_transpose, _x_bufs, tuple(_pool_ots),
           _pool_planes)
    if key not in _CACHED:
        _CACHED[key] = build_nc(mm_f32r=_mm_f32r, bin_dtype=_bin_dtype,
                                fuse_transpose=_fuse_transpose,
                                x_bufs=_x_bufs, pool_ots=tuple(_pool_ots),
                                pool_planes=_pool_planes)
    nc = _CACHED[key]
    in_maps = _stage_inputs(input, binary, alpha, bias, bin_dtype=_bin_dtype)
    res = run_bass_kernel_spmd(nc, in_maps, core_ids=list(range(N_CORES)),
                               trace=_trace)
    out = np.concatenate([res.results[c]["out"] for c in range(N_CORES)],
                         axis=1).reshape(B, S, O).astype(np.float32)
    if _trace:
        kernel.last_result = res
    return out



# revision 23
# speedup vs baseline: 1.3631x; 1.3631x over previous
"""BCQLinear packed forward on 8 Trainium2 NeuronCores.

Column-parallel sharding: binary/alpha/bias sharded along out_features
(4096 -> 8 x 512); activations replicated; host concatenates o-shards.

Per-core pipeline (fp8 DoubleRow formulation):
  W[o,i] = sum_b alpha[o,g,b] * B_b[o,i]   (i = 128 g + a)
  out    = x @ W^T + bias

  1. DVE/GPSIMD scale the +-1 bit-planes by alpha into bf16 planes
     (o on partitions), streamed per g-pair kk = (g0,g1).
  2. PE transpose-accumulates the 3 planes per [128,128] tile into a
     bf16 PSUM tile -> true W^T chunk [a, o].
  3. ACT casts the chunk to fp8e4 (wt_hi); DVE/GPSIMD write the
     residual to fp8e5 (wt_lo = W^T - wt_hi).  x is split on the host:
     x_hi = e4(x), x_lo = e5(x - x_hi).
  4. PE runs DoubleRow fp8 matmuls (contract 2 k-tiles/instruction):
     3 legs per (m, kk): x_hi*W_hi + x_lo*W_hi + x_hi*W_lo.
     A window of m-blocks accumulates in PSUM while dequant streams
     (wavefront); remaining m-blocks run as a pure-matmul tail.
  5. DVE adds bias, casts to bf16, DMA out.

Shapes hardcoded for this instance:
  input [2,1024,4096] f32; binary [4096,32,128,3] f32 (+-1);
  alpha [4096,32,3] f32; bias [4096] f32.
"""

import numpy as np
from contextlib import ExitStack

import bass_rust
import concourse.bass as bass
import concourse.mybir as mybir
import concourse.tile as tile
from concourse.bass_utils import run_bass_kernel_spmd
from concourse.masks import make_identity


def _legalize_waits(nc, max_waits=1):
    """Walrus allows only one sync-wait on (at least) DVE TensorTensor
    instructions. Move excess waits onto injected same-engine NoOps placed
    immediately before the instruction."""
    seq = 0
    for fn in nc.m.functions:
        for blk in fn.blocks:
            new_insts = []
            changed = False
            for inst in blk.instructions:
                si = inst.sync_info
                if si is not None and len(si.on_wait) > max_waits:
                    waits = list(si.on_wait)
                    for w in waits[:-max_waits]:
                        nop = mybir.InstNoOp(name=f"wlegal-{seq}")
                        seq += 1
                        nop.engine = inst.engine
                        nop.sync_info = bass_rust.SyncInfo(
                            on_wait=[w], on_update=[])
                        new_insts.append(nop)
                    inst.sync_info = bass_rust.SyncInfo(
                        on_wait=waits[-max_waits:],
                        on_update=list(si.on_update))
                    changed = True
                new_insts.append(inst)
            if changed:
                blk.instructions = new_insts


P = 128          # partitions
N_CORES = 8
B, S = 2, 1024
MS = B * S       # 2048 tokens
I = 4096         # in_features
O = 4096         # out_features
O_SH = O // N_CORES  # 512 per core
G, A, NB = 32, 128, 3
KK = G // 2      # 16 g-pairs (DoubleRow contracts 2 k-tiles)
MB = MS // P     # 16 token blocks
OT = O_SH // P   # 4 o-tiles per core

F32 = mybir.dt.float32
F32R = mybir.dt.float32r
BF16 = mybir.dt.bfloat16
E4 = mybir.dt.float8e4
E5 = mybir.dt.float8e5
DR = mybir.MatmulPerfMode.DoubleRow

_CACHED = {}


def build_nc(window: int = 6, admit_off: int = 3, admit_num: int = 17,
             admit_den: int = 20, x_bufs: int = 16, bq_bufs: int = 4,
             plane_bufs: int = 9, plane_dt: str = "bf16", bq_step: int = 1,
             wlo_dt: str = "e4", xlo_dt: str = "e4", tr_bufs: int = 2, xw_split: bool = False,
             out_bufs: int = 2, x2_start: int = 6, x2_queue: str = "sp", x2_mark: bool = False, scale_alt: bool = False,
             scale_gps_bits=(2,), sub_gps_ts=()) -> bass.Bass:
    nc = bass.Bass("TRN2", target_bir_lowering=False, debug=False)
    PDT = {"bf16": BF16, "f32r": F32R}[plane_dt]
    WLO = {"e5": E5, "e4": E4}[wlo_dt]
    XLO = {"e5": E5, "e4": E4}[xlo_dt]

    # Host-staged layouts (pure relayouts/casts of the sharded inputs):
    #  xhi/xlo [MB, P, KK, 2, P]: x[m*128+j, (2kk+t)*128+p] fp8 hi/lo split
    #  bq  [KK, P, OT, NB, 2, A]: binary[ot*128+p(o), g=2kk+t, a, b]
    #  al  [P, OT, G, NB]       : alpha[ot*128+p, g, b]
    #  biasr [P, O_SH]          : bias replicated across partitions
    xhi_d = nc.dram_tensor("xhi", [MB, P, KK, 2, P], E4, kind="ExternalInput").ap()
    xlo_d = nc.dram_tensor("xlo", [MB, P, KK, 2, P], XLO, kind="ExternalInput").ap()
    # bq grouped in chunks of bq_step g-pairs per DMA
    bq_d = nc.dram_tensor("bq", [KK, P, OT, NB, 2, A], E4, kind="ExternalInput").ap()
    al_d = nc.dram_tensor("al", [P, OT, G, NB], F32, kind="ExternalInput").ap()
    bias_d = nc.dram_tensor("biasr", [P, O_SH], F32, kind="ExternalInput").ap()
    out_d = nc.dram_tensor("out", [MS, O_SH], BF16, kind="ExternalOutput").ap()
    out_t = out_d.rearrange("(mb p) o -> mb p o", p=P)

    mult = mybir.AluOpType.mult
    add = mybir.AluOpType.add
    sub = mybir.AluOpType.subtract

    # static admission schedule for the m-block wavefront
    admits = [[] for _ in range(KK)]
    adm = 0
    for s in range(KK):
        want = min(window, admit_off + (admit_num * s) // admit_den)
        while adm < want:
            admits[s].append(adm)
            adm += 1
    while adm < window:
        admits[KK - 1].append(adm)
        adm += 1

    with tile.TileContext(nc) as tc, ExitStack() as ctx:
        const = ctx.enter_context(tc.tile_pool(name="const", bufs=1))
        wtp = ctx.enter_context(tc.tile_pool(name="wt", bufs=1))
        bqp = ctx.enter_context(tc.tile_pool(name="bq", bufs=bq_bufs))
        plp = ctx.enter_context(tc.tile_pool(name="pl", bufs=plane_bufs))
        xph = ctx.enter_context(tc.tile_pool(name="xh", bufs=x_bufs))
        xpl = ctx.enter_context(tc.tile_pool(name="xl", bufs=x_bufs))
        outp = ctx.enter_context(tc.tile_pool(name="out", bufs=out_bufs))
        ps_mm = ctx.enter_context(tc.tile_pool(name="psmm", bufs=window, space="PSUM"))
        ps_tr = ctx.enter_context(tc.tile_pool(name="pstr", bufs=tr_bufs, space="PSUM"))

        ident_f = const.tile([P, P], F32)
        make_identity(nc, ident_f)
        ident = const.tile([P, P], BF16, tag="identp")
        nc.vector.tensor_copy(ident, ident_f)
        al_sb = const.tile([P, OT, G, NB], F32, tag="al")
        bias_sb = const.tile([P, O_SH], F32, tag="bias")
        nc.sync.dma_start(al_sb, al_d)

        wt_hi = wtp.tile([P, G, O_SH], E4, tag="hi")
        wt_lo = wtp.tile([P, G, O_SH], WLO, tag="lo")

        # ---- all input DMAs, interleaved so x m-blocks arrive while the
        # binary g-pair chunks stream
        bq_sb, xhi_sb, xlo_sb = {}, {}, {}

        # bq/al stream on the ACT DGE queue so the small per-step binary
        # chunks are not head-of-line blocked behind the big x transfers
        # on the SP queue
        bq_dc = bq_d.rearrange("(c g) p ot nb t a -> c p g ot nb t a", g=bq_step)

        def load_bq(c):
            tile_ = bqp.tile([P, bq_step, OT, NB, 2, A], E4, name="bqt")
            for g in range(bq_step):
                bq_sb[c * bq_step + g] = tile_[:, g]
            nc.scalar.dma_start(tile_, bq_dc[c])

        # DMA order: bq chunks interleaved with only the window m-block x
        # loads (phase-1 critical), then the remaining bq burst, then the
        # phase-2 x stream. bq rides the ACT DGE queue, x the SP queue.
        def load_x(m):
            xhi_sb[m] = xph.tile([P, KK, 2, P], E4, name="xht")
            nc.sync.dma_start(xhi_sb[m], xhi_d[m])
            xlo_sb[m] = xpl.tile([P, KK, 2, P], XLO, name="xlt")
            nc.sync.dma_start(xlo_sb[m], xlo_d[m])

        NC = KK // bq_step
        nbq = min(3, NC)
        for c in range(nbq):
            load_bq(c)
        if xw_split:
            KH = KK // 2
            for m in range(window):
                xhi_sb[m] = xph.tile([P, KK, 2, P], E4, name="xht")
                xlo_sb[m] = xpl.tile([P, KK, 2, P], XLO, name="xlt")
                nc.sync.dma_start(xhi_sb[m][:, :KH], xhi_d[m][:, :KH])
                nc.sync.dma_start(xlo_sb[m][:, :KH], xlo_d[m][:, :KH])
                if nbq < NC:
                    load_bq(nbq)
                    nbq += 1
            while nbq < NC:
                load_bq(nbq)
                nbq += 1
            for m in range(window):
                nc.sync.dma_start(xhi_sb[m][:, KH:], xhi_d[m][:, KH:])
                nc.sync.dma_start(xlo_sb[m][:, KH:], xlo_d[m][:, KH:])
        else:
            for m in range(window):
                load_x(m)
                if nbq < NC:
                    load_bq(nbq)
                    nbq += 1
            while nbq < NC:
                load_bq(nbq)
                nbq += 1
        nc.scalar.dma_start(bias_sb, bias_d)
        # phase-2 x loads are issued from the DVE DGE queue inside the
        # step loop (paced by compute) so they do not crowd the DMA
        # device while the phase-1-critical bq/x-window transfers stream
        xq2 = list(range(window, MB))
        for m in xq2:
            xhi_sb[m] = xph.tile([P, KK, 2, P], E4, name="xht")
            xlo_sb[m] = xpl.tile([P, KK, 2, P], XLO, name="xlt")
            if x2_queue in ("act", "sp"):
                q = nc.scalar if x2_queue == "act" else nc.sync
                if x2_mark:
                    nc.gpsimd.memset(xhi_sb[m][:, 0, 0, :4], 0)
                    nc.gpsimd.memset(xlo_sb[m][:, 0, 0, :4], 0)
                q.dma_start(xhi_sb[m], xhi_d[m])
                q.dma_start(xlo_sb[m], xlo_d[m])

        # ---- per-step state
        planes = {}   # (s, b) -> bf16 plane tile [P, OT, 2, A]
        trs = {}      # (s, t) -> psum tile
        ps_of = {}    # m -> psum accumulator
        next_kk = {}  # m -> next kk to emit legs for
        nleg = {m: 0 for m in range(MB)}
        out_done = set()

        def scale(s, b):
            gbits = scale_gps_bits if (not scale_alt or s % 2 == 0) else (1, 2)
            eng = nc.gpsimd if b in gbits else nc.vector
            pl = plp.tile([P, OT, 2, A], PDT, name="plt")
            planes[(s, b)] = pl
            eng.tensor_tensor(
                pl, bq_sb[s][:, :, b],
                al_sb[:, :, 2 * s:2 * s + 2, b:b + 1].to_broadcast([P, OT, 2, A]),
                mult)

        def transposes(s):
            # transpose-accumulate via a REGULAR matmul against a constant
            # identity (out[a,o] = sum_o' plane[o',a] I[o',o]): same cost
            # as is_transpose (1.0 cyc/row keyed on the moving identity)
            # but uses the standard f32 PSUM accumulation path, which is
            # what real HW supports for multi-plane accumulation.
            for t in range(2):
                ps = ps_tr.tile([P, OT * P], F32, name="trt")
                trs[(s, t)] = ps
                for ot in range(OT):
                    for b in range(NB):
                        nc.tensor.matmul(
                            ps[:, ot * P:(ot + 1) * P],
                            planes[(s, b)][:, ot, t], ident,
                            start=(ot == 0 and b == 0),
                            stop=(ot == OT - 1 and b == NB - 1))

        def cast_hi(s, t):
            nc.scalar.copy(wt_hi[:, 2 * s + t, :], trs[(s, t)])

        def sub_lo(s, t):
            eng = nc.gpsimd if t in sub_gps_ts else nc.vector
            eng.tensor_tensor(wt_lo[:, 2 * s + t, :], trs[(s, t)],
                              wt_hi[:, 2 * s + t, :], sub)

        def leg(m, lhsT, rhs):
            nc.tensor.matmul(ps_of[m], lhsT, rhs,
                             start=(nleg[m] == 0),
                             stop=(nleg[m] == 3 * KK - 1),
                             perf_mode=DR)
            nleg[m] += 1

        def legs_hi(m, s):
            rhs_hi = wt_hi[:, 2 * s:2 * s + 2, :]
            leg(m, xhi_sb[m][:, s], rhs_hi)
            leg(m, xlo_sb[m][:, s], rhs_hi)

        def legs_lo(m, s):
            leg(m, xhi_sb[m][:, s], wt_lo[:, 2 * s:2 * s + 2, :])

        def finalize(m):
            out_sb = outp.tile([P, O_SH], BF16, name="ob")
            nc.vector.tensor_tensor(out_sb, ps_of[m], bias_sb, add)
            nc.sync.dma_start(out_t[m], out_sb)
            out_done.add(m)

        # ---- phase 1: stream dequant, wavefront of window m-blocks.
        # Per step s: PE runs hi-legs for kk=s-1, lo-legs for kk=s-2 (one
        # extra step of slack for the wt_lo residual), then transposes(s).
        # DVE/GPSIMD run this step's scales before last step's residual
        # subs; ACT casts trail the transposes.
        nhi = {}  # m -> next kk for hi legs
        nlo = {}  # m -> next kk for lo legs

        def emit_pe_legs(s):
            # interleave hi(kk=s-1) and lo(kk=s-2) legs per m
            for m in sorted(nhi):
                while nhi[m] < min(s - 1, KK):
                    legs_hi(m, nhi[m])
                    nhi[m] += 1
                while nlo[m] < min(s - 1, KK):
                    legs_lo(m, nlo[m])
                    nlo[m] += 1

        nx2 = 0
        for s in range(KK + 2):
            if x2_queue == "gps" and s >= x2_start and nx2 < len(xq2):
                m = xq2[nx2]
                nc.gpsimd.dma_start(xhi_sb[m], xhi_d[m])
                nc.gpsimd.dma_start(xlo_sb[m], xlo_d[m])
                nx2 += 1
            if s < KK:
                for b in range(NB):
                    scale(s, b)
            if 1 <= s <= KK:
                for t in range(2):
                    sub_lo(s - 1, t)
            emit_pe_legs(s)
            if s < KK:
                transposes(s)
                for t in range(2):
                    cast_hi(s, t)
                for m in admits[s]:
                    ps_of[m] = ps_mm.tile([P, O_SH], F32, name="acc")
                    nhi[m] = 0
                    nlo[m] = 0
        for m in sorted(nhi):
            assert nhi[m] == KK and nlo[m] == KK and nleg[m] == 3 * KK
            finalize(m)

        if x2_queue == "gps":
            while nx2 < len(xq2):
                m = xq2[nx2]
                nc.gpsimd.dma_start(xhi_sb[m], xhi_d[m])
                nc.gpsimd.dma_start(xlo_sb[m], xlo_d[m])
                nx2 += 1

        # ---- phase 2: remaining m-blocks, pure matmul
        for m in range(window, MB):
            ps_of[m] = ps_mm.tile([P, O_SH], F32, name="acc")
            nhi[m] = nlo[m] = 0
            for s in range(KK):
                legs_hi(m, s)
                legs_lo(m, s)
            finalize(m)

    _legalize_waits(nc)
    return nc


def _stage_inputs(input, binary, alpha, bias, xlo_dt="e4"):
    np_e4 = mybir.dt.np(E4)
    np_xlo = mybir.dt.np({"e5": E5, "e4": E4}[xlo_dt])

    x = np.ascontiguousarray(np.asarray(input, dtype=np.float32)).reshape(MS, I)
    x_hi = x.astype(np_e4)
    x_lo = (x - x_hi.astype(np.float32)).astype(np_xlo)
    # [MS, I] -> [m, j, kk, t, p] -> [m, p, kk, t, j]
    def relayout_x(a):
        return np.ascontiguousarray(
            a.reshape(MB, P, KK, 2, P).transpose(0, 4, 2, 3, 1))
    xhi = relayout_x(x_hi)
    xlo = relayout_x(x_lo)

    binary = np.asarray(binary, dtype=np.float32)
    alpha = np.ascontiguousarray(np.asarray(alpha, dtype=np.float32))
    bias = np.asarray(bias, dtype=np.float32)

    in_maps = []
    for c in range(N_CORES):
        sl = slice(c * O_SH, (c + 1) * O_SH)
        # binary [512, G, A, NB] -> [ot, p, kk, t, a, b] -> [kk, p, ot, b, t, a]
        bc = binary[sl].reshape(OT, P, KK, 2, A, NB)
        bq = np.ascontiguousarray(bc.transpose(2, 1, 0, 5, 3, 4)).astype(np_e4)
        al = np.ascontiguousarray(
            alpha[sl].reshape(OT, P, G, NB).transpose(1, 0, 2, 3))
        in_maps.append({
            "xhi": xhi,
            "xlo": xlo,
            "bq": bq,
            "al": al,
            "biasr": np.ascontiguousarray(
                np.broadcast_to(bias[sl][None, :], (P, O_SH))),
        })
    return in_maps


def kernel(input, binary, alpha, bias, _trace=False, **cfg):
    key = tuple(sorted(cfg.items()))
    if key not in _CACHED:
        _CACHED[key] = build_nc(**cfg)
    nc = _CACHED[key]
    in_maps = _stage_inputs(input, binary, alpha, bias,
                            xlo_dt=cfg.get("xlo_dt", "e4"))
    res = run_bass_kernel_spmd(nc, in_maps, core_ids=list(range(N_CORES)),
                               trace=_trace)
    out = np.concatenate(
        [np.asarray(res.results[c]["out"]) for c in range(N_CORES)],
        axis=1).astype(np.float32).reshape(B, S, O)
    kernel.last_result = res
    return out


# revision 26
# speedup vs baseline: 1.3697x; 1.0049x over previous
"""BCQLinear packed forward on 8 Trainium2 NeuronCores.

Column-parallel sharding: binary/alpha/bias sharded along out_features
(4096 -> 8 x 512); activations replicated; host concatenates o-shards.

Per-core pipeline (fp8 DoubleRow formulation):
  W[o,i] = sum_b alpha[o,g,b] * B_b[o,i]   (i = 128 g + a)
  out    = x @ W^T + bias

  1. DVE/GPSIMD scale the +-1 bit-planes by alpha into bf16 planes
     (o on partitions), streamed per g-pair kk = (g0,g1).
  2. PE transpose-accumulates the 3 planes per [128,128] tile into a
     bf16 PSUM tile -> true W^T chunk [a, o].
  3. ACT casts the chunk to fp8e4 (wt_hi); DVE/GPSIMD write the
     residual to fp8e5 (wt_lo = W^T - wt_hi).  x is split on the host:
     x_hi = e4(x), x_lo = e5(x - x_hi).
  4. PE runs DoubleRow fp8 matmuls (contract 2 k-tiles/instruction):
     3 legs per (m, kk): x_hi*W_hi + x_lo*W_hi + x_hi*W_lo.
     A window of m-blocks accumulates in PSUM while dequant streams
     (wavefront); remaining m-blocks run as a pure-matmul tail.
  5. DVE adds bias, casts to bf16, DMA out.

Shapes hardcoded for this instance:
  input [2,1024,4096] f32; binary [4096,32,128,3] f32 (+-1);
  alpha [4096,32,3] f32; bias [4096] f32.
"""

import numpy as np
from contextlib import ExitStack

import bass_rust
import concourse.bass as bass
import concourse.mybir as mybir
import concourse.tile as tile
from concourse.bass_utils import run_bass_kernel_spmd
from concourse.masks import make_identity


def _legalize_waits(nc, max_waits=1):
    """Walrus allows only one sync-wait on (at least) DVE TensorTensor
    instructions. Move excess waits onto injected same-engine NoOps placed
    immediately before the instruction."""
    seq = 0
    for fn in nc.m.functions:
        for blk in fn.blocks:
            new_insts = []
            changed = False
            for inst in blk.instructions:
                si = inst.sync_info
                if si is not None and len(si.on_wait) > max_waits:
                    waits = list(si.on_wait)
                    for w in waits[:-max_waits]:
                        nop = mybir.InstNoOp(name=f"wlegal-{seq}")
                        seq += 1
                        nop.engine = inst.engine
                        nop.sync_info = bass_rust.SyncInfo(
                            on_wait=[w], on_update=[])
                        new_insts.append(nop)
                    inst.sync_info = bass_rust.SyncInfo(
                        on_wait=waits[-max_waits:],
                        on_update=list(si.on_update))
                    changed = True
                new_insts.append(inst)
            if changed:
                blk.instructions = new_insts


P = 128          # partitions
N_CORES = 8
B, S = 2, 1024
MS = B * S       # 2048 tokens
I = 4096         # in_features
O = 4096         # out_features
O_SH = O // N_CORES  # 512 per core
G, A, NB = 32, 128, 3
KK = G // 2      # 16 g-pairs (DoubleRow contracts 2 k-tiles)
MB = MS // P     # 16 token blocks
OT = O_SH // P   # 4 o-tiles per core

F32 = mybir.dt.float32
F32R = mybir.dt.float32r
BF16 = mybir.dt.bfloat16
E4 = mybir.dt.float8e4
E5 = mybir.dt.float8e5
DR = mybir.MatmulPerfMode.DoubleRow

_CACHED = {}


def build_nc(window: int = 6, admit_off: int = 3, admit_num: int = 17,
             admit_den: int = 20, x_bufs: int = 16, bq_bufs: int = 4,
             plane_bufs: int = 9, plane_dt: str = "bf16", bq_step: int = 1,
             wlo_dt: str = "e4", xlo_dt: str = "e4", tr_bufs: int = 2, xw_split: bool = False,
             out_bufs: int = 2, x2_start: int = 6, x2_queue: str = "sp", x2_mark: bool = False, scale_alt: bool = False, sub_split: int = 0, scale_tsplit_bit: int = -1,
             scale_gps_bits=(2,), sub_gps_ts=()) -> bass.Bass:
    nc = bass.Bass("TRN2", target_bir_lowering=False, debug=False)
    PDT = {"bf16": BF16, "f32r": F32R}[plane_dt]
    WLO = {"e5": E5, "e4": E4}[wlo_dt]
    XLO = {"e5": E5, "e4": E4}[xlo_dt]

    # Host-staged layouts (pure relayouts/casts of the sharded inputs):
    #  xhi/xlo [MB, P, KK, 2, P]: x[m*128+j, (2kk+t)*128+p] fp8 hi/lo split
    #  bq  [KK, P, OT, NB, 2, A]: binary[ot*128+p(o), g=2kk+t, a, b]
    #  al  [P, OT, G, NB]       : alpha[ot*128+p, g, b]
    #  biasr [P, O_SH]          : bias replicated across partitions
    xhi_d = nc.dram_tensor("xhi", [MB, P, KK, 2, P], E4, kind="ExternalInput").ap()
    xlo_d = nc.dram_tensor("xlo", [MB, P, KK, 2, P], XLO, kind="ExternalInput").ap()
    # bq grouped in chunks of bq_step g-pairs per DMA
    bq_d = nc.dram_tensor("bq", [KK, P, OT, NB, 2, A], E4, kind="ExternalInput").ap()
    al_d = nc.dram_tensor("al", [P, OT, G, NB], F32, kind="ExternalInput").ap()
    bias_d = nc.dram_tensor("biasr", [P, O_SH], F32, kind="ExternalInput").ap()
    out_d = nc.dram_tensor("out", [MS, O_SH], BF16, kind="ExternalOutput").ap()
    out_t = out_d.rearrange("(mb p) o -> mb p o", p=P)

    mult = mybir.AluOpType.mult
    add = mybir.AluOpType.add
    sub = mybir.AluOpType.subtract

    # static admission schedule for the m-block wavefront
    admits = [[] for _ in range(KK)]
    adm = 0
    for s in range(KK):
        want = min(window, admit_off + (admit_num * s) // admit_den)
        while adm < want:
            admits[s].append(adm)
            adm += 1
    while adm < window:
        admits[KK - 1].append(adm)
        adm += 1

    with tile.TileContext(nc) as tc, ExitStack() as ctx:
        const = ctx.enter_context(tc.tile_pool(name="const", bufs=1))
        wtp = ctx.enter_context(tc.tile_pool(name="wt", bufs=1))
        bqp = ctx.enter_context(tc.tile_pool(name="bq", bufs=bq_bufs))
        plp = ctx.enter_context(tc.tile_pool(name="pl", bufs=plane_bufs))
        xph = ctx.enter_context(tc.tile_pool(name="xh", bufs=x_bufs))
        xpl = ctx.enter_context(tc.tile_pool(name="xl", bufs=x_bufs))
        outp = ctx.enter_context(tc.tile_pool(name="out", bufs=out_bufs))
        ps_mm = ctx.enter_context(tc.tile_pool(name="psmm", bufs=window, space="PSUM"))
        ps_tr = ctx.enter_context(tc.tile_pool(name="pstr", bufs=tr_bufs, space="PSUM"))

        ident_f = const.tile([P, P], F32)
        make_identity(nc, ident_f)
        ident = const.tile([P, P], BF16, tag="identp")
        nc.vector.tensor_copy(ident, ident_f)
        al_sb = const.tile([P, OT, G, NB], F32, tag="al")
        bias_sb = const.tile([P, O_SH], F32, tag="bias")
        nc.sync.dma_start(al_sb, al_d)

        wt_hi = wtp.tile([P, G, O_SH], E4, tag="hi")
        wt_lo = wtp.tile([P, G, O_SH], WLO, tag="lo")

        # ---- all input DMAs, interleaved so x m-blocks arrive while the
        # binary g-pair chunks stream
        bq_sb, xhi_sb, xlo_sb = {}, {}, {}

        # bq/al stream on the ACT DGE queue so the small per-step binary
        # chunks are not head-of-line blocked behind the big x transfers
        # on the SP queue
        bq_dc = bq_d.rearrange("(c g) p ot nb t a -> c p g ot nb t a", g=bq_step)

        def load_bq(c):
            tile_ = bqp.tile([P, bq_step, OT, NB, 2, A], E4, name="bqt")
            for g in range(bq_step):
                bq_sb[c * bq_step + g] = tile_[:, g]
            # first chunk rides the lower-latency SP queue (startup)
            q = nc.sync if c == 0 else nc.scalar
            q.dma_start(tile_, bq_dc[c])

        # DMA order: bq chunks interleaved with only the window m-block x
        # loads (phase-1 critical), then the remaining bq burst, then the
        # phase-2 x stream. bq rides the ACT DGE queue, x the SP queue.
        def load_x(m):
            xhi_sb[m] = xph.tile([P, KK, 2, P], E4, name="xht")
            nc.sync.dma_start(xhi_sb[m], xhi_d[m])
            xlo_sb[m] = xpl.tile([P, KK, 2, P], XLO, name="xlt")
            nc.sync.dma_start(xlo_sb[m], xlo_d[m])

        NC = KK // bq_step
        nbq = min(3, NC)
        for c in range(nbq):
            load_bq(c)
        if xw_split:
            KH = KK // 2
            for m in range(window):
                xhi_sb[m] = xph.tile([P, KK, 2, P], E4, name="xht")
                xlo_sb[m] = xpl.tile([P, KK, 2, P], XLO, name="xlt")
                nc.sync.dma_start(xhi_sb[m][:, :KH], xhi_d[m][:, :KH])
                nc.sync.dma_start(xlo_sb[m][:, :KH], xlo_d[m][:, :KH])
                if nbq < NC:
                    load_bq(nbq)
                    nbq += 1
            while nbq < NC:
                load_bq(nbq)
                nbq += 1
            for m in range(window):
                nc.sync.dma_start(xhi_sb[m][:, KH:], xhi_d[m][:, KH:])
                nc.sync.dma_start(xlo_sb[m][:, KH:], xlo_d[m][:, KH:])
        else:
            for m in range(window):
                load_x(m)
                if nbq < NC:
                    load_bq(nbq)
                    nbq += 1
            while nbq < NC:
                load_bq(nbq)
                nbq += 1
        nc.scalar.dma_start(bias_sb, bias_d)
        # phase-2 x loads are issued from the DVE DGE queue inside the
        # step loop (paced by compute) so they do not crowd the DMA
        # device while the phase-1-critical bq/x-window transfers stream
        xq2 = list(range(window, MB))
        for m in xq2:
            xhi_sb[m] = xph.tile([P, KK, 2, P], E4, name="xht")
            xlo_sb[m] = xpl.tile([P, KK, 2, P], XLO, name="xlt")
            if x2_queue in ("act", "sp"):
                q = nc.scalar if x2_queue == "act" else nc.sync
                if x2_mark:
                    nc.gpsimd.memset(xhi_sb[m][:, 0, 0, :4], 0)
                    nc.gpsimd.memset(xlo_sb[m][:, 0, 0, :4], 0)
                q.dma_start(xhi_sb[m], xhi_d[m])
                q.dma_start(xlo_sb[m], xlo_d[m])

        # ---- per-step state
        planes = {}   # (s, b) -> bf16 plane tile [P, OT, 2, A]
        trs = {}      # (s, t) -> psum tile
        ps_of = {}    # m -> psum accumulator
        next_kk = {}  # m -> next kk to emit legs for
        nleg = {m: 0 for m in range(MB)}
        out_done = set()

        def scale(s, b):
            gbits = scale_gps_bits if (not scale_alt or s % 2 == 0) else (1, 2)
            pl = plp.tile([P, OT, 2, A], PDT, name="plt")
            planes[(s, b)] = pl
            if b == scale_tsplit_bit:
                # split this bit-plane's scale by t-half across DVE/GPSIMD
                for t, eng in ((0, nc.vector), (1, nc.gpsimd)):
                    eng.tensor_tensor(
                        pl[:, :, t], bq_sb[s][:, :, b, t],
                        al_sb[:, :, 2 * s + t, b:b + 1].to_broadcast([P, OT, A]),
                        mult)
                return
            eng = nc.gpsimd if b in gbits else nc.vector
            eng.tensor_tensor(
                pl, bq_sb[s][:, :, b],
                al_sb[:, :, 2 * s:2 * s + 2, b:b + 1].to_broadcast([P, OT, 2, A]),
                mult)

        def transposes(s):
            # transpose-accumulate via a REGULAR matmul against a constant
            # identity (out[a,o] = sum_o' plane[o',a] I[o',o]): same cost
            # as is_transpose (1.0 cyc/row keyed on the moving identity)
            # but uses the standard f32 PSUM accumulation path, which is
            # what real HW supports for multi-plane accumulation.
            for t in range(2):
                ps = ps_tr.tile([P, OT * P], F32, name="trt")
                trs[(s, t)] = ps
                for ot in range(OT):
                    for b in range(NB):
                        nc.tensor.matmul(
                            ps[:, ot * P:(ot + 1) * P],
                            planes[(s, b)][:, ot, t], ident,
                            start=(ot == 0 and b == 0),
                            stop=(ot == OT - 1 and b == NB - 1))

        def cast_hi(s, t):
            nc.scalar.copy(wt_hi[:, 2 * s + t, :], trs[(s, t)])

        def sub_lo(s, t):
            if sub_split and t == 1:
                h = sub_split
                nc.vector.tensor_tensor(wt_lo[:, 2 * s + t, :h],
                                        trs[(s, t)][:, :h],
                                        wt_hi[:, 2 * s + t, :h], sub)
                nc.gpsimd.tensor_tensor(wt_lo[:, 2 * s + t, h:],
                                        trs[(s, t)][:, h:],
                                        wt_hi[:, 2 * s + t, h:], sub)
                return
            eng = nc.gpsimd if t in sub_gps_ts else nc.vector
            eng.tensor_tensor(wt_lo[:, 2 * s + t, :], trs[(s, t)],
                              wt_hi[:, 2 * s + t, :], sub)

        def leg(m, lhsT, rhs):
            nc.tensor.matmul(ps_of[m], lhsT, rhs,
                             start=(nleg[m] == 0),
                             stop=(nleg[m] == 3 * KK - 1),
                             perf_mode=DR)
            nleg[m] += 1

        def legs_hi(m, s):
            rhs_hi = wt_hi[:, 2 * s:2 * s + 2, :]
            leg(m, xhi_sb[m][:, s], rhs_hi)
            leg(m, xlo_sb[m][:, s], rhs_hi)

        def legs_lo(m, s):
            leg(m, xhi_sb[m][:, s], wt_lo[:, 2 * s:2 * s + 2, :])

        def finalize(m):
            out_sb = outp.tile([P, O_SH], BF16, name="ob")
            nc.vector.tensor_tensor(out_sb, ps_of[m], bias_sb, add)
            nc.sync.dma_start(out_t[m], out_sb)
            out_done.add(m)

        # ---- phase 1: stream dequant, wavefront of window m-blocks.
        # Per step s: PE runs hi-legs for kk=s-1, lo-legs for kk=s-2 (one
        # extra step of slack for the wt_lo residual), then transposes(s).
        # DVE/GPSIMD run this step's scales before last step's residual
        # subs; ACT casts trail the transposes.
        nhi = {}  # m -> next kk for hi legs
        nlo = {}  # m -> next kk for lo legs

        def emit_pe_legs(s):
            # interleave hi(kk=s-1) and lo(kk=s-2) legs per m
            for m in sorted(nhi):
                while nhi[m] < min(s - 1, KK):
                    legs_hi(m, nhi[m])
                    nhi[m] += 1
                while nlo[m] < min(s - 1, KK):
                    legs_lo(m, nlo[m])
                    nlo[m] += 1

        nx2 = 0
        for s in range(KK + 2):
            if x2_queue == "gps" and s >= x2_start and nx2 < len(xq2):
                m = xq2[nx2]
                nc.gpsimd.dma_start(xhi_sb[m], xhi_d[m])
                nc.gpsimd.dma_start(xlo_sb[m], xlo_d[m])
                nx2 += 1
            if s < KK:
                for b in range(NB):
                    scale(s, b)
            if 1 <= s <= KK:
                for t in range(2):
                    sub_lo(s - 1, t)
            emit_pe_legs(s)
            if s < KK:
                transposes(s)
                for t in range(2):
                    cast_hi(s, t)
                for m in admits[s]:
                    ps_of[m] = ps_mm.tile([P, O_SH], F32, name="acc")
                    nhi[m] = 0
                    nlo[m] = 0
        for m in sorted(nhi):
            assert nhi[m] == KK and nlo[m] == KK and nleg[m] == 3 * KK
            finalize(m)

        if x2_queue == "gps":
            while nx2 < len(xq2):
                m = xq2[nx2]
                nc.gpsimd.dma_start(xhi_sb[m], xhi_d[m])
                nc.gpsimd.dma_start(xlo_sb[m], xlo_d[m])
                nx2 += 1

        # ---- phase 2: remaining m-blocks, pure matmul
        for m in range(window, MB):
            ps_of[m] = ps_mm.tile([P, O_SH], F32, name="acc")
            nhi[m] = nlo[m] = 0
            for s in range(KK):
                legs_hi(m, s)
                legs_lo(m, s)
            finalize(m)

    _legalize_waits(nc)
    return nc


def _stage_inputs(input, binary, alpha, bias, xlo_dt="e4"):
    np_e4 = mybir.dt.np(E4)
    np_xlo = mybir.dt.np({"e5": E5, "e4": E4}[xlo_dt])

    x = np.ascontiguousarray(np.asarray(input, dtype=np.float32)).reshape(MS, I)
    x_hi = x.astype(np_e4)
    x_lo = (x - x_hi.astype(np.float32)).astype(np_xlo)
    # [MS, I] -> [m, j, kk, t, p] -> [m, p, kk, t, j]
    def relayout_x(a):
        return np.ascontiguousarray(
            a.reshape(MB, P, KK, 2, P).transpose(0, 4, 2, 3, 1))
    xhi = relayout_x(x_hi)
    xlo = relayout_x(x_lo)

    binary = np.asarray(binary, dtype=np.float32)
    alpha = np.ascontiguousarray(np.asarray(alpha, dtype=np.float32))
    bias = np.asarray(bias, dtype=np.float32)

    in_maps = []
    for c in range(N_CORES):
        sl = slice(c * O_SH, (c + 1) * O_SH)
        # binary [512, G, A, NB] -> [ot, p, kk, t, a, b] -> [kk, p, ot, b, t, a]
        bc = binary[sl].reshape(OT, P, KK, 2, A, NB)
        bq = np.ascontiguousarray(bc.transpose(2, 1, 0, 5, 3, 4)).astype(np_e4)
        al = np.ascontiguousarray(
            alpha[sl].reshape(OT, P, G, NB).transpose(1, 0, 2, 3))
        in_maps.append({
            "xhi": xhi,
            "xlo": xlo,
            "bq": bq,
            "al": al,
            "biasr": np.ascontiguousarray(
                np.broadcast_to(bias[sl][None, :], (P, O_SH))),
        })
    return in_maps


def kernel(input, binary, alpha, bias, _trace=False, **cfg):
    key = tuple(sorted(cfg.items()))
    if key not in _CACHED:
        _CACHED[key] = build_nc(**cfg)
    nc = _CACHED[key]
    in_maps = _stage_inputs(input, binary, alpha, bias,
                            xlo_dt=cfg.get("xlo_dt", "e4"))
    res = run_bass_kernel_spmd(nc, in_maps, core_ids=list(range(N_CORES)),
                               trace=_trace)
    out = np.concatenate(
        [np.asarray(res.results[c]["out"]) for c in range(N_CORES)],
        axis=1).astype(np.float32).reshape(B, S, O)
    kernel.last_result = res
    return out


# revision 39
# speedup vs baseline: 1.3714x; 1.0012x over previous
"""BCQLinear packed forward on 8 Trainium2 NeuronCores.

Column-parallel sharding: binary/alpha/bias sharded along out_features
(4096 -> 8 x 512); activations replicated; host concatenates o-shards.

Per-core pipeline (fp8 DoubleRow formulation):
  W[o,i] = sum_b alpha[o,g,b] * B_b[o,i]   (i = 128 g + a)
  out    = x @ W^T + bias

  1. DVE/GPSIMD scale the +-1 bit-planes by alpha into bf16 planes
     (o on partitions), streamed per g-pair kk = (g0,g1).
  2. PE transpose-accumulates the 3 planes per [128,128] tile into a
     bf16 PSUM tile -> true W^T chunk [a, o].
  3. ACT casts the chunk to fp8e4 (wt_hi); DVE/GPSIMD write the
     residual to fp8e5 (wt_lo = W^T - wt_hi).  x is split on the host:
     x_hi = e4(x), x_lo = e5(x - x_hi).
  4. PE runs DoubleRow fp8 matmuls (contract 2 k-tiles/instruction):
     3 legs per (m, kk): x_hi*W_hi + x_lo*W_hi + x_hi*W_lo.
     A window of m-blocks accumulates in PSUM while dequant streams
     (wavefront); remaining m-blocks run as a pure-matmul tail.
  5. DVE adds bias, casts to bf16, DMA out.

Shapes hardcoded for this instance:
  input [2,1024,4096] f32; binary [4096,32,128,3] f32 (+-1);
  alpha [4096,32,3] f32; bias [4096] f32.
"""

import numpy as np
from contextlib import ExitStack

import bass_rust
import concourse.bass as bass
import concourse.mybir as mybir
import concourse.tile as tile
from concourse.bass_utils import run_bass_kernel_spmd
from concourse.masks import make_identity


def _legalize_waits(nc, max_waits=1):
    """Walrus allows only one sync-wait on (at least) DVE TensorTensor
    instructions. Move excess waits onto injected same-engine NoOps placed
    immediately before the instruction."""
    seq = 0
    for fn in nc.m.functions:
        for blk in fn.blocks:
            new_insts = []
            changed = False
            for inst in blk.instructions:
                si = inst.sync_info
                if si is not None and len(si.on_wait) > max_waits:
                    waits = list(si.on_wait)
                    for w in waits[:-max_waits]:
                        nop = mybir.InstNoOp(name=f"wlegal-{seq}")
                        seq += 1
                        nop.engine = inst.engine
                        nop.sync_info = bass_rust.SyncInfo(
                            on_wait=[w], on_update=[])
                        new_insts.append(nop)
                    inst.sync_info = bass_rust.SyncInfo(
                        on_wait=waits[-max_waits:],
                        on_update=list(si.on_update))
                    changed = True
                new_insts.append(inst)
            if changed:
                blk.instructions = new_insts


P = 128          # partitions
N_CORES = 8
B, S = 2, 1024
MS = B * S       # 2048 tokens
I = 4096         # in_features
O = 4096         # out_features
O_SH = O // N_CORES  # 512 per core
G, A, NB = 32, 128, 3
KK = G // 2      # 16 g-pairs (DoubleRow contracts 2 k-tiles)
MB = MS // P     # 16 token blocks
OT = O_SH // P   # 4 o-tiles per core

F32 = mybir.dt.float32
F32R = mybir.dt.float32r
BF16 = mybir.dt.bfloat16
E4 = mybir.dt.float8e4
E5 = mybir.dt.float8e5
DR = mybir.MatmulPerfMode.DoubleRow

_CACHED = {}


def build_nc(window: int = 6, admit_off: int = 3, admit_num: int = 17,
             admit_den: int = 20, x_bufs: int = 16, bq_bufs: int = 4,
             plane_bufs: int = 9, plane_dt: str = "bf16", bq_step: int = 1,
             wlo_dt: str = "e4", xlo_dt: str = "e4", tr_bufs: int = 2, xw_split: bool = False,
             out_bufs: int = 2, x2_start: int = 6, x2_queue: str = "sp", x2_mark: bool = False, scale_alt: bool = False, sub_split: int = 0, scale_tsplit_bit: int = -1, s0_tsplit: int = 0, ramp_flip: int = 0, sub_sched: str = "", spill_mode: bool = False,
             spill1: int = 14, reload1: int = 15, burst: int = 3, prt_bufs: int = 8,
             scale_gps_bits=(2,), sub_gps_ts=()) -> bass.Bass:
    nc = bass.Bass("TRN2", target_bir_lowering=False, debug=False)
    PDT = {"bf16": BF16, "f32r": F32R}[plane_dt]
    WLO = {"e5": E5, "e4": E4}[wlo_dt]
    XLO = {"e5": E5, "e4": E4}[xlo_dt]

    # Host-staged layouts (pure relayouts/casts of the sharded inputs):
    #  xhi/xlo [MB, P, KK, 2, P]: x[m*128+j, (2kk+t)*128+p] fp8 hi/lo split
    #  bq  [KK, P, OT, NB, 2, A]: binary[ot*128+p(o), g=2kk+t, a, b]
    #  al  [P, OT, G, NB]       : alpha[ot*128+p, g, b]
    #  biasr [P, O_SH]          : bias replicated across partitions
    xhi_d = nc.dram_tensor("xhi", [MB, P, KK, 2, P], E4, kind="ExternalInput").ap()
    xlo_d = nc.dram_tensor("xlo", [MB, P, KK, 2, P], XLO, kind="ExternalInput").ap()
    # bq grouped in chunks of bq_step g-pairs per DMA
    bq_d = nc.dram_tensor("bq", [KK, P, OT, NB, 2, A], E4, kind="ExternalInput").ap()
    al_d = nc.dram_tensor("al", [P, OT, G, NB], F32, kind="ExternalInput").ap()
    bias_d = nc.dram_tensor("biasr", [P, O_SH], F32, kind="ExternalInput").ap()
    out_d = nc.dram_tensor("out", [MS, O_SH], BF16, kind="ExternalOutput").ap()
    out_t = out_d.rearrange("(mb p) o -> mb p o", p=P)

    mult = mybir.AluOpType.mult
    add = mybir.AluOpType.add
    sub = mybir.AluOpType.subtract

    # static admission schedule for the m-block wavefront
    admits = [[] for _ in range(KK)]
    adm = 0
    for s in range(KK):
        want = min(window, admit_off + (admit_num * s) // admit_den)
        while adm < want:
            admits[s].append(adm)
            adm += 1
    while adm < window:
        admits[KK - 1].append(adm)
        adm += 1

    with tile.TileContext(nc) as tc, ExitStack() as ctx:
        const = ctx.enter_context(tc.tile_pool(name="const", bufs=1))
        wtp = ctx.enter_context(tc.tile_pool(name="wt", bufs=1))
        bqp = ctx.enter_context(tc.tile_pool(name="bq", bufs=bq_bufs))
        plp = ctx.enter_context(tc.tile_pool(name="pl", bufs=plane_bufs))
        xph = ctx.enter_context(tc.tile_pool(name="xh", bufs=x_bufs))
        xpl = ctx.enter_context(tc.tile_pool(name="xl", bufs=x_bufs))
        outp = ctx.enter_context(tc.tile_pool(name="out", bufs=out_bufs))
        prtp = ctx.enter_context(tc.tile_pool(name="prt", bufs=prt_bufs))
        ps_mm = ctx.enter_context(tc.tile_pool(name="psmm", bufs=window, space="PSUM"))
        ps_tr = ctx.enter_context(tc.tile_pool(name="pstr", bufs=tr_bufs, space="PSUM"))

        ident_f = const.tile([P, P], F32)
        make_identity(nc, ident_f)
        ident = const.tile([P, P], BF16, tag="identp")
        nc.vector.tensor_copy(ident, ident_f)
        al_sb = const.tile([P, OT, G, NB], F32, tag="al")
        bias_sb = const.tile([P, O_SH], F32, tag="bias")
        nc.sync.dma_start(al_sb, al_d)

        wt_hi = wtp.tile([P, G, O_SH], E4, tag="hi")
        wt_lo = wtp.tile([P, G, O_SH], WLO, tag="lo")

        # ---- all input DMAs, interleaved so x m-blocks arrive while the
        # binary g-pair chunks stream
        bq_sb, xhi_sb, xlo_sb = {}, {}, {}

        # bq/al stream on the ACT DGE queue so the small per-step binary
        # chunks are not head-of-line blocked behind the big x transfers
        # on the SP queue
        bq_dc = bq_d.rearrange("(c g) p ot nb t a -> c p g ot nb t a", g=bq_step)

        def load_bq(c):
            tile_ = bqp.tile([P, bq_step, OT, NB, 2, A], E4, name="bqt")
            for g in range(bq_step):
                bq_sb[c * bq_step + g] = tile_[:, g]
            # first chunk rides the lower-latency SP queue (startup)
            q = nc.sync if c == 0 else nc.scalar
            q.dma_start(tile_, bq_dc[c])

        # DMA order: bq chunks interleaved with only the window m-block x
        # loads (phase-1 critical), then the remaining bq burst, then the
        # phase-2 x stream. bq rides the ACT DGE queue, x the SP queue.
        def load_x(m):
            xhi_sb[m] = xph.tile([P, KK, 2, P], E4, name="xht")
            nc.sync.dma_start(xhi_sb[m], xhi_d[m])
            xlo_sb[m] = xpl.tile([P, KK, 2, P], XLO, name="xlt")
            nc.sync.dma_start(xlo_sb[m], xlo_d[m])

        NC = KK // bq_step
        nbq = min(3, NC)
        for c in range(nbq):
            load_bq(c)
        if xw_split:
            KH = KK // 2
            for m in range(window):
                xhi_sb[m] = xph.tile([P, KK, 2, P], E4, name="xht")
                xlo_sb[m] = xpl.tile([P, KK, 2, P], XLO, name="xlt")
                nc.sync.dma_start(xhi_sb[m][:, :KH], xhi_d[m][:, :KH])
                nc.sync.dma_start(xlo_sb[m][:, :KH], xlo_d[m][:, :KH])
                if nbq < NC:
                    load_bq(nbq)
                    nbq += 1
            while nbq < NC:
                load_bq(nbq)
                nbq += 1
            for m in range(window):
                nc.sync.dma_start(xhi_sb[m][:, KH:], xhi_d[m][:, KH:])
                nc.sync.dma_start(xlo_sb[m][:, KH:], xlo_d[m][:, KH:])
        else:
            for m in range(window):
                load_x(m)
                if nbq < NC:
                    load_bq(nbq)
                    nbq += 1
            while nbq < NC:
                load_bq(nbq)
                nbq += 1
        nc.scalar.dma_start(bias_sb, bias_d)
        # phase-2 x loads are issued from the DVE DGE queue inside the
        # step loop (paced by compute) so they do not crowd the DMA
        # device while the phase-1-critical bq/x-window transfers stream
        xq2 = list(range(window, MB))
        for m in xq2:
            xhi_sb[m] = xph.tile([P, KK, 2, P], E4, name="xht")
            xlo_sb[m] = xpl.tile([P, KK, 2, P], XLO, name="xlt")
            if x2_queue in ("act", "sp"):
                q = nc.scalar if x2_queue == "act" else nc.sync
                if x2_mark:
                    nc.gpsimd.memset(xhi_sb[m][:, 0, 0, :4], 0)
                    nc.gpsimd.memset(xlo_sb[m][:, 0, 0, :4], 0)
                q.dma_start(xhi_sb[m], xhi_d[m])
                q.dma_start(xlo_sb[m], xlo_d[m])

        # ---- per-step state
        part_sb = {}  # m -> spilled partial (bf16)
        planes = {}   # (s, b) -> bf16 plane tile [P, OT, 2, A]
        trs = {}      # (s, t) -> psum tile
        ps_of = {}    # m -> psum accumulator
        next_kk = {}  # m -> next kk to emit legs for
        nleg = {m: 0 for m in range(MB)}
        out_done = set()

        def scale(s, b):
            gbits = scale_gps_bits if (not scale_alt or s % 2 == 0) else (1, 2)
            pl = plp.tile([P, OT, 2, A], PDT, name="plt")
            planes[(s, b)] = pl
            if b == scale_tsplit_bit:
                # split this bit-plane's scale by t-half across DVE/GPSIMD
                for t, eng in ((0, nc.vector), (1, nc.gpsimd)):
                    eng.tensor_tensor(
                        pl[:, :, t], bq_sb[s][:, :, b, t],
                        al_sb[:, :, 2 * s + t, b:b + 1].to_broadcast([P, OT, A]),
                        mult)
                return
            eng = nc.gpsimd if b in gbits else nc.vector
            eng.tensor_tensor(
                pl, bq_sb[s][:, :, b],
                al_sb[:, :, 2 * s:2 * s + 2, b:b + 1].to_broadcast([P, OT, 2, A]),
                mult)

        def transposes(s):
            # transpose-accumulate via a REGULAR matmul against a constant
            # identity (out[a,o] = sum_o' plane[o',a] I[o',o]): same cost
            # as is_transpose (1.0 cyc/row keyed on the moving identity)
            # but uses the standard f32 PSUM accumulation path, which is
            # what real HW supports for multi-plane accumulation.
            for t in range(2):
                ps = ps_tr.tile([P, OT * P], F32, name="trt")
                trs[(s, t)] = ps
                for ot in range(OT):
                    for b in range(NB):
                        nc.tensor.matmul(
                            ps[:, ot * P:(ot + 1) * P],
                            planes[(s, b)][:, ot, t], ident,
                            start=(ot == 0 and b == 0),
                            stop=(ot == OT - 1 and b == NB - 1))

        def cast_hi(s, t):
            nc.scalar.copy(wt_hi[:, 2 * s + t, :], trs[(s, t)])

        def sub_lo(s, t):
            if sub_split and t == 1:
                h = sub_split
                nc.vector.tensor_tensor(wt_lo[:, 2 * s + t, :h],
                                        trs[(s, t)][:, :h],
                                        wt_hi[:, 2 * s + t, :h], sub)
                nc.gpsimd.tensor_tensor(wt_lo[:, 2 * s + t, h:],
                                        trs[(s, t)][:, h:],
                                        wt_hi[:, 2 * s + t, h:], sub)
                return
            eng = nc.gpsimd if t in sub_gps_ts else nc.vector
            eng.tensor_tensor(wt_lo[:, 2 * s + t, :], trs[(s, t)],
                              wt_hi[:, 2 * s + t, :], sub)

        ten_first = {}  # m -> True if current tenancy is fresh (start leg)
        ten_last = {}   # m -> kk bound of current tenancy (exclusive)
        ten_skip = {}   # m -> skip group check (headless reload tenancy)

        def leg(m, lhsT, rhs, is_last):
            nc.tensor.matmul(ps_of[m], lhsT, rhs,
                             start=ten_first.pop(m, False),
                             stop=is_last,
                             perf_mode=DR,
                             skip_group_check=ten_skip.get(m, False))
            nleg[m] += 1

        def legs_hi(m, s):
            rhs_hi = wt_hi[:, 2 * s:2 * s + 2, :]
            leg(m, xhi_sb[m][:, s], rhs_hi, False)
            leg(m, xlo_sb[m][:, s], rhs_hi, False)

        def legs_lo(m, s):
            # lo leg is always the tenancy's last emitted leg for kk s
            leg(m, xhi_sb[m][:, s], wt_lo[:, 2 * s:2 * s + 2, :],
                s == ten_last[m] - 1)

        def finalize(m):
            out_sb = outp.tile([P, O_SH], BF16, name="ob")
            part = part_sb.pop(m, None)
            if part is not None:
                tmp = outp.tile([P, O_SH], F32, tag="tmpf", name="tf")
                nc.vector.tensor_tensor(tmp, ps_of[m], bias_sb, add)
                nc.vector.tensor_tensor(out_sb, tmp, part, add)
            else:
                nc.vector.tensor_tensor(out_sb, ps_of[m], bias_sb, add)
            nc.sync.dma_start(out_t[m], out_sb)
            out_done.add(m)

        # ---- phase 1: stream dequant, wavefront of window m-blocks.
        # Per step s: PE runs hi-legs for kk=s-1, lo-legs for kk=s-2 (one
        # extra step of slack for the wt_lo residual), then transposes(s).
        # DVE/GPSIMD run this step's scales before last step's residual
        # subs; ACT casts trail the transposes.
        nhi = {}  # m -> next kk for hi legs
        nlo = {}  # m -> next kk for lo legs

        def emit_pe_legs(s):
            # interleave hi(kk=s-1) and lo(kk=s-2) legs per m
            for m in sorted(ps_of):
                lim = min(s - 1, cap[m],
                          nhi[m] + burst if s > rbase[m] else nhi[m])
                while nhi[m] < lim:
                    legs_hi(m, nhi[m])
                    nhi[m] += 1
                while nlo[m] < min(lim, s - lo_lag):
                    legs_lo(m, nlo[m])
                    nlo[m] += 1

        nx2 = 0
        lo_lag = 2 if sub_sched == "stag" else 1
        cap = {}      # m -> tenancy kk bound
        rbase = {}    # m -> catch-up ramp base step
        KH = KK // 2

        def open_ten(m, kk0, kk1, base_s, fresh):
            ps_of[m] = ps_mm.tile([P, O_SH], F32, name="acc")
            nhi[m] = nlo[m] = kk0
            cap[m] = ten_last[m] = kk1
            rbase[m] = base_s
            # every tenancy is a fresh accumulation group; spilled partials
            # are merged back at finalize (PSUM preload + headless
            # accumulation does not work on real HW)
            ten_first[m] = True
            ten_skip[m] = not fresh

        def spill(m):
            part_sb[m] = prtp.tile([P, O_SH], BF16, name="part")
            nc.scalar.copy(part_sb[m], ps_of[m])
            del ps_of[m]

        for s in range(KK + 1 + lo_lag):
            if x2_queue == "gps" and s >= x2_start and nx2 < len(xq2):
                m = xq2[nx2]
                nc.gpsimd.dma_start(xhi_sb[m], xhi_d[m])
                nc.gpsimd.dma_start(xlo_sb[m], xlo_d[m])
                nx2 += 1
            if sub_sched != "stag" and 1 <= s <= KK and s <= ramp_flip:
                # ramp: produce W_lo before the next scales so PE's lo-legs
                # are not starved while the pipeline fills
                for t in range(2):
                    sub_lo(s - 1, t)
            if s < KK:
                if s < s0_tsplit:
                    # pipeline-fill: emit all bit-planes as t-halves with t
                    # outer so the first transpose group's inputs finish
                    # ~2us earlier
                    for b in range(NB):
                        planes[(s, b)] = plp.tile([P, OT, 2, A], PDT,
                                                  name="plt")
                    for t in range(2):
                        for b in range(NB):
                            eng = (nc.gpsimd if b in scale_gps_bits
                                   else nc.vector)
                            eng.tensor_tensor(
                                planes[(s, b)][:, :, t],
                                bq_sb[s][:, :, b, t],
                                al_sb[:, :, 2 * s + t, b:b + 1]
                                .to_broadcast([P, OT, A]), mult)
                else:
                    for b in range(NB):
                        scale(s, b)
            if sub_sched == "stag" and 1 <= s <= KK:
                nc.vector.tensor_tensor(
                    wt_lo[:, 2 * (s - 1), :], trs[(s - 1, 0)],
                    wt_hi[:, 2 * (s - 1), :], sub)
            if sub_sched == "stag":
                # staggered residuals: t0 on DVE one step stale (emitted
                # after the scales below), t1 on GPSIMD two steps stale so
                # neither engine waits on a fresh cast
                if 2 <= s <= KK + 1:
                    nc.gpsimd.tensor_tensor(
                        wt_lo[:, 2 * (s - 2) + 1, :], trs[(s - 2, 1)],
                        wt_hi[:, 2 * (s - 2) + 1, :], sub)
            elif 1 <= s <= KK and s > ramp_flip:
                for t in range(2):
                    sub_lo(s - 1, t)
            emit_pe_legs(s)
            if s < KK:
                transposes(s)
                for t in range(2):
                    cast_hi(s, t)
                for m in admits[s]:
                    open_ten(m, 0, KH if spill_mode else KK, 0, True)
                if spill_mode and s == spill1:
                    for m in range(window):
                        spill(m)
                    for m in range(window, 2 * window):
                        open_ten(m, 0, KH, s, True)
                if spill_mode and s == reload1:
                    for m in range(window, 2 * window):
                        spill(m)
                    for m in range(window):
                        open_ten(m, KH, KK, s, False)
        for m in sorted(ps_of):
            assert cap[m] == KK
            while nhi[m] < KK:
                legs_hi(m, nhi[m])
                nhi[m] += 1
            while nlo[m] < KK:
                legs_lo(m, nlo[m])
                nlo[m] += 1
            finalize(m)

        if x2_queue == "gps":
            while nx2 < len(xq2):
                m = xq2[nx2]
                nc.gpsimd.dma_start(xhi_sb[m], xhi_d[m])
                nc.gpsimd.dma_start(xlo_sb[m], xlo_d[m])
                nx2 += 1

        # ---- phase 2: remaining m-blocks
        if spill_mode:
            for m in range(window, 2 * window):
                open_ten(m, KH, KK, 0, False)
                rbase.pop(m, None)
                for s in range(KH, KK):
                    legs_hi(m, s)
                    legs_lo(m, s)
                finalize(m)
            rest = range(2 * window, MB)
        else:
            rest = range(window, MB)
        for m in rest:
            open_ten(m, 0, KK, 0, True)
            rbase.pop(m, None)
            for s in range(KK):
                legs_hi(m, s)
                legs_lo(m, s)
            finalize(m)

    _legalize_waits(nc)
    return nc


def _stage_inputs(input, binary, alpha, bias, xlo_dt="e4"):
    np_e4 = mybir.dt.np(E4)
    np_xlo = mybir.dt.np({"e5": E5, "e4": E4}[xlo_dt])

    x = np.ascontiguousarray(np.asarray(input, dtype=np.float32)).reshape(MS, I)
    x_hi = x.astype(np_e4)
    x_lo = (x - x_hi.astype(np.float32)).astype(np_xlo)
    # [MS, I] -> [m, j, kk, t, p] -> [m, p, kk, t, j]
    def relayout_x(a):
        return np.ascontiguousarray(
            a.reshape(MB, P, KK, 2, P).transpose(0, 4, 2, 3, 1))
    xhi = relayout_x(x_hi)
    xlo = relayout_x(x_lo)

    binary = np.asarray(binary, dtype=np.float32)
    alpha = np.ascontiguousarray(np.asarray(alpha, dtype=np.float32))
    bias = np.asarray(bias, dtype=np.float32)

    in_maps = []
    for c in range(N_CORES):
        sl = slice(c * O_SH, (c + 1) * O_SH)
        # binary [512, G, A, NB] -> [ot, p, kk, t, a, b] -> [kk, p, ot, b, t, a]
        bc = binary[sl].reshape(OT, P, KK, 2, A, NB)
        bq = np.ascontiguousarray(bc.transpose(2, 1, 0, 5, 3, 4)).astype(np_e4)
        al = np.ascontiguousarray(
            alpha[sl].reshape(OT, P, G, NB).transpose(1, 0, 2, 3))
        in_maps.append({
            "xhi": xhi,
            "xlo": xlo,
            "bq": bq,
            "al": al,
            "biasr": np.ascontiguousarray(
                np.broadcast_to(bias[sl][None, :], (P, O_SH))),
        })
    return in_maps


def kernel(input, binary, alpha, bias, _trace=False, **cfg):
    key = tuple(sorted(cfg.items()))
    if key not in _CACHED:
        _CACHED[key] = build_nc(**cfg)
    nc = _CACHED[key]
    in_maps = _stage_inputs(input, binary, alpha, bias,
                            xlo_dt=cfg.get("xlo_dt", "e4"))
    res = run_bass_kernel_spmd(nc, in_maps, core_ids=list(range(N_CORES)),
                               trace=_trace)
    out = np.concatenate(
        [np.asarray(res.results[c]["out"]) for c in range(N_CORES)],
        axis=1).astype(np.float32).reshape(B, S, O)
    kernel.last_result = res
    return out


# revision 41
# speedup vs baseline: 1.3798x; 1.0061x over previous
"""BCQLinear packed forward on 8 Trainium2 NeuronCores.

Column-parallel sharding: binary/alpha/bias sharded along out_features
(4096 -> 8 x 512); activations replicated; host concatenates o-shards.

Per-core pipeline (fp8 DoubleRow formulation):
  W[o,i] = sum_b alpha[o,g,b] * B_b[o,i]   (i = 128 g + a)
  out    = x @ W^T + bias

  1. DVE/GPSIMD scale the +-1 bit-planes by alpha into bf16 planes
     (o on partitions), streamed per g-pair kk = (g0,g1).
  2. PE transpose-accumulates the 3 planes per [128,128] tile into a
     bf16 PSUM tile -> true W^T chunk [a, o].
  3. ACT casts the chunk to fp8e4 (wt_hi); DVE/GPSIMD write the
     residual to fp8e5 (wt_lo = W^T - wt_hi).  x is split on the host:
     x_hi = e4(x), x_lo = e5(x - x_hi).
  4. PE runs DoubleRow fp8 matmuls (contract 2 k-tiles/instruction):
     3 legs per (m, kk): x_hi*W_hi + x_lo*W_hi + x_hi*W_lo.
     A window of m-blocks accumulates in PSUM while dequant streams
     (wavefront); remaining m-blocks run as a pure-matmul tail.
  5. DVE adds bias, casts to bf16, DMA out.

Shapes hardcoded for this instance:
  input [2,1024,4096] f32; binary [4096,32,128,3] f32 (+-1);
  alpha [4096,32,3] f32; bias [4096] f32.
"""

import numpy as np
from contextlib import ExitStack

import bass_rust
import concourse.bass as bass
import concourse.mybir as mybir
import concourse.tile as tile
from concourse.bass_utils import run_bass_kernel_spmd
from concourse.masks import make_identity


def _legalize_waits(nc, max_waits=1):
    """Walrus allows only one sync-wait on (at least) DVE TensorTensor
    instructions. Move excess waits onto injected same-engine NoOps placed
    immediately before the instruction."""
    seq = 0
    for fn in nc.m.functions:
        for blk in fn.blocks:
            new_insts = []
            changed = False
            for inst in blk.instructions:
                si = inst.sync_info
                if si is not None and len(si.on_wait) > max_waits:
                    waits = list(si.on_wait)
                    for w in waits[:-max_waits]:
                        nop = mybir.InstNoOp(name=f"wlegal-{seq}")
                        seq += 1
                        nop.engine = inst.engine
                        nop.sync_info = bass_rust.SyncInfo(
                            on_wait=[w], on_update=[])
                        new_insts.append(nop)
                    inst.sync_info = bass_rust.SyncInfo(
                        on_wait=waits[-max_waits:],
                        on_update=list(si.on_update))
                    changed = True
                new_insts.append(inst)
            if changed:
                blk.instructions = new_insts


P = 128          # partitions
N_CORES = 8
B, S = 2, 1024
MS = B * S       # 2048 tokens
I = 4096         # in_features
O = 4096         # out_features
O_SH = O // N_CORES  # 512 per core
G, A, NB = 32, 128, 3
KK = G // 2      # 16 g-pairs (DoubleRow contracts 2 k-tiles)
MB = MS // P     # 16 token blocks
OT = O_SH // P   # 4 o-tiles per core

F32 = mybir.dt.float32
F32R = mybir.dt.float32r
BF16 = mybir.dt.bfloat16
E4 = mybir.dt.float8e4
E5 = mybir.dt.float8e5
DR = mybir.MatmulPerfMode.DoubleRow

_CACHED = {}


def build_nc(window: int = 6, admit_off: int = 3, admit_num: int = 17,
             admit_den: int = 20, x_bufs: int = 15, bq_bufs: int = 4,
             plane_bufs: int = 9, plane_dt: str = "bf16", bq_step: int = 1,
             wlo_dt: str = "e4", xlo_dt: str = "e4", tr_bufs: int = 2, xw_split: bool = False,
             out_bufs: int = 2, x2_start: int = 6, x2_queue: str = "sp", x2_mark: bool = False, scale_alt: bool = False, sub_split: int = 0, scale_tsplit_bit: int = -1, s0_tsplit: int = 0, ramp_flip: int = 0, sub_sched: str = "", spill_mode: bool = True,
             spill1: int = 12, reload1: int = 15, burst: int = 3, prt_bufs: int = 8,
             scale_gps_bits=(2,), sub_gps_ts=()) -> bass.Bass:
    nc = bass.Bass("TRN2", target_bir_lowering=False, debug=False)
    PDT = {"bf16": BF16, "f32r": F32R}[plane_dt]
    WLO = {"e5": E5, "e4": E4}[wlo_dt]
    XLO = {"e5": E5, "e4": E4}[xlo_dt]

    # Host-staged layouts (pure relayouts/casts of the sharded inputs):
    #  xhi/xlo [MB, P, KK, 2, P]: x[m*128+j, (2kk+t)*128+p] fp8 hi/lo split
    #  bq  [KK, P, OT, NB, 2, A]: binary[ot*128+p(o), g=2kk+t, a, b]
    #  al  [P, OT, G, NB]       : alpha[ot*128+p, g, b]
    #  biasr [P, O_SH]          : bias replicated across partitions
    xhi_d = nc.dram_tensor("xhi", [MB, P, KK, 2, P], E4, kind="ExternalInput").ap()
    xlo_d = nc.dram_tensor("xlo", [MB, P, KK, 2, P], XLO, kind="ExternalInput").ap()
    # bq grouped in chunks of bq_step g-pairs per DMA
    bq_d = nc.dram_tensor("bq", [KK, P, OT, NB, 2, A], E4, kind="ExternalInput").ap()
    al_d = nc.dram_tensor("al", [P, OT, G, NB], F32, kind="ExternalInput").ap()
    bias_d = nc.dram_tensor("biasr", [P, O_SH], F32, kind="ExternalInput").ap()
    out_d = nc.dram_tensor("out", [MS, O_SH], BF16, kind="ExternalOutput").ap()
    out_t = out_d.rearrange("(mb p) o -> mb p o", p=P)

    mult = mybir.AluOpType.mult
    add = mybir.AluOpType.add
    sub = mybir.AluOpType.subtract

    # static admission schedule for the m-block wavefront
    admits = [[] for _ in range(KK)]
    adm = 0
    for s in range(KK):
        want = min(window, admit_off + (admit_num * s) // admit_den)
        while adm < want:
            admits[s].append(adm)
            adm += 1
    while adm < window:
        admits[KK - 1].append(adm)
        adm += 1

    with tile.TileContext(nc) as tc, ExitStack() as ctx:
        const = ctx.enter_context(tc.tile_pool(name="const", bufs=1))
        wtp = ctx.enter_context(tc.tile_pool(name="wt", bufs=1))
        bqp = ctx.enter_context(tc.tile_pool(name="bq", bufs=bq_bufs))
        plp = ctx.enter_context(tc.tile_pool(name="pl", bufs=plane_bufs))
        xph = ctx.enter_context(tc.tile_pool(name="xh", bufs=x_bufs))
        xpl = ctx.enter_context(tc.tile_pool(name="xl", bufs=x_bufs))
        outp = ctx.enter_context(tc.tile_pool(name="out", bufs=out_bufs))
        prtp = ctx.enter_context(tc.tile_pool(name="prt", bufs=prt_bufs))
        ps_mm = ctx.enter_context(tc.tile_pool(name="psmm", bufs=window, space="PSUM"))
        ps_tr = ctx.enter_context(tc.tile_pool(name="pstr", bufs=tr_bufs, space="PSUM"))

        ident_f = const.tile([P, P], F32)
        make_identity(nc, ident_f)
        ident = const.tile([P, P], BF16, tag="identp")
        nc.vector.tensor_copy(ident, ident_f)
        al_sb = const.tile([P, OT, G, NB], F32, tag="al")
        bias_sb = const.tile([P, O_SH], F32, tag="bias")
        nc.sync.dma_start(al_sb, al_d)

        wt_hi = wtp.tile([P, G, O_SH], E4, tag="hi")
        wt_lo = wtp.tile([P, G, O_SH], WLO, tag="lo")

        # ---- all input DMAs, interleaved so x m-blocks arrive while the
        # binary g-pair chunks stream
        bq_sb, xhi_sb, xlo_sb = {}, {}, {}

        # bq/al stream on the ACT DGE queue so the small per-step binary
        # chunks are not head-of-line blocked behind the big x transfers
        # on the SP queue
        bq_dc = bq_d.rearrange("(c g) p ot nb t a -> c p g ot nb t a", g=bq_step)

        def load_bq(c):
            tile_ = bqp.tile([P, bq_step, OT, NB, 2, A], E4, name="bqt")
            for g in range(bq_step):
                bq_sb[c * bq_step + g] = tile_[:, g]
            # first chunk rides the lower-latency SP queue (startup)
            q = nc.sync if c == 0 else nc.scalar
            q.dma_start(tile_, bq_dc[c])

        # DMA order: bq chunks interleaved with only the window m-block x
        # loads (phase-1 critical), then the remaining bq burst, then the
        # phase-2 x stream. bq rides the ACT DGE queue, x the SP queue.
        def load_x(m):
            xhi_sb[m] = xph.tile([P, KK, 2, P], E4, name="xht")
            nc.sync.dma_start(xhi_sb[m], xhi_d[m])
            xlo_sb[m] = xpl.tile([P, KK, 2, P], XLO, name="xlt")
            nc.sync.dma_start(xlo_sb[m], xlo_d[m])

        NC = KK // bq_step
        nbq = min(3, NC)
        for c in range(nbq):
            load_bq(c)
        if xw_split:
            KH = KK // 2
            for m in range(window):
                xhi_sb[m] = xph.tile([P, KK, 2, P], E4, name="xht")
                xlo_sb[m] = xpl.tile([P, KK, 2, P], XLO, name="xlt")
                nc.sync.dma_start(xhi_sb[m][:, :KH], xhi_d[m][:, :KH])
                nc.sync.dma_start(xlo_sb[m][:, :KH], xlo_d[m][:, :KH])
                if nbq < NC:
                    load_bq(nbq)
                    nbq += 1
            while nbq < NC:
                load_bq(nbq)
                nbq += 1
            for m in range(window):
                nc.sync.dma_start(xhi_sb[m][:, KH:], xhi_d[m][:, KH:])
                nc.sync.dma_start(xlo_sb[m][:, KH:], xlo_d[m][:, KH:])
        else:
            for m in range(window):
                load_x(m)
                if nbq < NC:
                    load_bq(nbq)
                    nbq += 1
            while nbq < NC:
                load_bq(nbq)
                nbq += 1
        nc.scalar.dma_start(bias_sb, bias_d)
        # phase-2 x loads are issued from the DVE DGE queue inside the
        # step loop (paced by compute) so they do not crowd the DMA
        # device while the phase-1-critical bq/x-window transfers stream
        xq2 = list(range(window, MB))
        for m in xq2:
            xhi_sb[m] = xph.tile([P, KK, 2, P], E4, name="xht")
            xlo_sb[m] = xpl.tile([P, KK, 2, P], XLO, name="xlt")
            if x2_queue in ("act", "sp"):
                q = nc.scalar if x2_queue == "act" else nc.sync
                if x2_mark:
                    nc.gpsimd.memset(xhi_sb[m][:, 0, 0, :4], 0)
                    nc.gpsimd.memset(xlo_sb[m][:, 0, 0, :4], 0)
                q.dma_start(xhi_sb[m], xhi_d[m])
                q.dma_start(xlo_sb[m], xlo_d[m])

        # ---- per-step state
        part_sb = {}  # m -> spilled partial (bf16)
        planes = {}   # (s, b) -> bf16 plane tile [P, OT, 2, A]
        trs = {}      # (s, t) -> psum tile
        ps_of = {}    # m -> psum accumulator
        next_kk = {}  # m -> next kk to emit legs for
        nleg = {m: 0 for m in range(MB)}
        out_done = set()

        def scale(s, b):
            gbits = scale_gps_bits if (not scale_alt or s % 2 == 0) else (1, 2)
            pl = plp.tile([P, OT, 2, A], PDT, name="plt")
            planes[(s, b)] = pl
            if b == scale_tsplit_bit:
                # split this bit-plane's scale by t-half across DVE/GPSIMD
                for t, eng in ((0, nc.vector), (1, nc.gpsimd)):
                    eng.tensor_tensor(
                        pl[:, :, t], bq_sb[s][:, :, b, t],
                        al_sb[:, :, 2 * s + t, b:b + 1].to_broadcast([P, OT, A]),
                        mult)
                return
            eng = nc.gpsimd if b in gbits else nc.vector
            eng.tensor_tensor(
                pl, bq_sb[s][:, :, b],
                al_sb[:, :, 2 * s:2 * s + 2, b:b + 1].to_broadcast([P, OT, 2, A]),
                mult)

        def transposes(s):
            # transpose-accumulate via a REGULAR matmul against a constant
            # identity (out[a,o] = sum_o' plane[o',a] I[o',o]): same cost
            # as is_transpose (1.0 cyc/row keyed on the moving identity)
            # but uses the standard f32 PSUM accumulation path, which is
            # what real HW supports for multi-plane accumulation.
            for t in range(2):
                ps = ps_tr.tile([P, OT * P], F32, name="trt")
                trs[(s, t)] = ps
                for ot in range(OT):
                    for b in range(NB):
                        nc.tensor.matmul(
                            ps[:, ot * P:(ot + 1) * P],
                            planes[(s, b)][:, ot, t], ident,
                            start=(ot == 0 and b == 0),
                            stop=(ot == OT - 1 and b == NB - 1))

        def cast_hi(s, t):
            nc.scalar.copy(wt_hi[:, 2 * s + t, :], trs[(s, t)])

        def sub_lo(s, t):
            if sub_split and t == 1:
                h = sub_split
                nc.vector.tensor_tensor(wt_lo[:, 2 * s + t, :h],
                                        trs[(s, t)][:, :h],
                                        wt_hi[:, 2 * s + t, :h], sub)
                nc.gpsimd.tensor_tensor(wt_lo[:, 2 * s + t, h:],
                                        trs[(s, t)][:, h:],
                                        wt_hi[:, 2 * s + t, h:], sub)
                return
            eng = nc.gpsimd if t in sub_gps_ts else nc.vector
            eng.tensor_tensor(wt_lo[:, 2 * s + t, :], trs[(s, t)],
                              wt_hi[:, 2 * s + t, :], sub)

        ten_first = {}  # m -> True if current tenancy is fresh (start leg)
        ten_last = {}   # m -> kk bound of current tenancy (exclusive)
        ten_skip = {}   # m -> skip group check (headless reload tenancy)

        def leg(m, lhsT, rhs, is_last):
            nc.tensor.matmul(ps_of[m], lhsT, rhs,
                             start=ten_first.pop(m, False),
                             stop=is_last,
                             perf_mode=DR,
                             skip_group_check=ten_skip.get(m, False))
            nleg[m] += 1

        def legs_hi(m, s):
            rhs_hi = wt_hi[:, 2 * s:2 * s + 2, :]
            leg(m, xhi_sb[m][:, s], rhs_hi, False)
            leg(m, xlo_sb[m][:, s], rhs_hi, False)

        def legs_lo(m, s):
            # lo leg is always the tenancy's last emitted leg for kk s
            leg(m, xhi_sb[m][:, s], wt_lo[:, 2 * s:2 * s + 2, :],
                s == ten_last[m] - 1)

        def finalize(m):
            out_sb = outp.tile([P, O_SH], BF16, name="ob")
            part = part_sb.pop(m, None)
            if part is not None:
                tmp = outp.tile([P, O_SH], F32, tag="tmpf", name="tf")
                nc.vector.tensor_tensor(tmp, ps_of[m], bias_sb, add)
                nc.vector.tensor_tensor(out_sb, tmp, part, add)
            else:
                nc.vector.tensor_tensor(out_sb, ps_of[m], bias_sb, add)
            nc.sync.dma_start(out_t[m], out_sb)
            out_done.add(m)

        # ---- phase 1: stream dequant, wavefront of window m-blocks.
        # Per step s: PE runs hi-legs for kk=s-1, lo-legs for kk=s-2 (one
        # extra step of slack for the wt_lo residual), then transposes(s).
        # DVE/GPSIMD run this step's scales before last step's residual
        # subs; ACT casts trail the transposes.
        nhi = {}  # m -> next kk for hi legs
        nlo = {}  # m -> next kk for lo legs

        def emit_pe_legs(s):
            # interleave hi(kk=s-1) and lo(kk=s-2) legs per m
            for m in sorted(ps_of):
                lim = min(s - 1, cap[m],
                          nhi[m] + burst if s > rbase[m] else nhi[m])
                while nhi[m] < lim:
                    legs_hi(m, nhi[m])
                    nhi[m] += 1
                while nlo[m] < min(lim, s - lo_lag):
                    legs_lo(m, nlo[m])
                    nlo[m] += 1

        nx2 = 0
        lo_lag = 2 if sub_sched == "stag" else 1
        cap = {}      # m -> tenancy kk bound
        rbase = {}    # m -> catch-up ramp base step
        KH = KK // 2

        def open_ten(m, kk0, kk1, base_s, fresh):
            ps_of[m] = ps_mm.tile([P, O_SH], F32, name="acc")
            nhi[m] = nlo[m] = kk0
            cap[m] = ten_last[m] = kk1
            rbase[m] = base_s
            # every tenancy is a fresh accumulation group; spilled partials
            # are merged back at finalize (PSUM preload + headless
            # accumulation does not work on real HW)
            ten_first[m] = True
            ten_skip[m] = not fresh

        def flush_ten(m):
            # emit every remaining leg of the current tenancy before the
            # accumulator is read/spilled -- legs not yet emitted here
            # would otherwise be silently dropped
            while nhi[m] < cap[m]:
                legs_hi(m, nhi[m])
                nhi[m] += 1
            while nlo[m] < cap[m]:
                legs_lo(m, nlo[m])
                nlo[m] += 1

        def spill(m):
            part_sb[m] = prtp.tile([P, O_SH], BF16, name="part")
            nc.scalar.copy(part_sb[m], ps_of[m])
            del ps_of[m]

        for s in range(KK + 1 + lo_lag):
            if x2_queue == "gps" and s >= x2_start and nx2 < len(xq2):
                m = xq2[nx2]
                nc.gpsimd.dma_start(xhi_sb[m], xhi_d[m])
                nc.gpsimd.dma_start(xlo_sb[m], xlo_d[m])
                nx2 += 1
            if sub_sched != "stag" and 1 <= s <= KK and s <= ramp_flip:
                # ramp: produce W_lo before the next scales so PE's lo-legs
                # are not starved while the pipeline fills
                for t in range(2):
                    sub_lo(s - 1, t)
            if s < KK:
                if s < s0_tsplit:
                    # pipeline-fill: emit all bit-planes as t-halves with t
                    # outer so the first transpose group's inputs finish
                    # ~2us earlier
                    for b in range(NB):
                        planes[(s, b)] = plp.tile([P, OT, 2, A], PDT,
                                                  name="plt")
                    for t in range(2):
                        for b in range(NB):
                            eng = (nc.gpsimd if b in scale_gps_bits
                                   else nc.vector)
                            eng.tensor_tensor(
                                planes[(s, b)][:, :, t],
                                bq_sb[s][:, :, b, t],
                                al_sb[:, :, 2 * s + t, b:b + 1]
                                .to_broadcast([P, OT, A]), mult)
                else:
                    for b in range(NB):
                        scale(s, b)
            if sub_sched == "stag" and 1 <= s <= KK:
                nc.vector.tensor_tensor(
                    wt_lo[:, 2 * (s - 1), :], trs[(s - 1, 0)],
                    wt_hi[:, 2 * (s - 1), :], sub)
            if sub_sched == "stag":
                # staggered residuals: t0 on DVE one step stale (emitted
                # after the scales below), t1 on GPSIMD two steps stale so
                # neither engine waits on a fresh cast
                if 2 <= s <= KK + 1:
                    nc.gpsimd.tensor_tensor(
                        wt_lo[:, 2 * (s - 2) + 1, :], trs[(s - 2, 1)],
                        wt_hi[:, 2 * (s - 2) + 1, :], sub)
            elif 1 <= s <= KK and s > ramp_flip:
                for t in range(2):
                    sub_lo(s - 1, t)
            emit_pe_legs(s)
            if s < KK:
                transposes(s)
                for t in range(2):
                    cast_hi(s, t)
                for m in admits[s]:
                    open_ten(m, 0, KH if spill_mode else KK, 0, True)
                if spill_mode and s == spill1:
                    for m in range(window):
                        flush_ten(m)
                        spill(m)
                    for m in range(window, 2 * window):
                        open_ten(m, 0, KH, s, True)
                if spill_mode and s == reload1:
                    for m in range(window, 2 * window):
                        flush_ten(m)
                        spill(m)
                    for m in range(window):
                        open_ten(m, KH, KK, s, False)
        for m in sorted(ps_of):
            assert cap[m] == KK
            while nhi[m] < KK:
                legs_hi(m, nhi[m])
                nhi[m] += 1
            while nlo[m] < KK:
                legs_lo(m, nlo[m])
                nlo[m] += 1
            finalize(m)

        if x2_queue == "gps":
            while nx2 < len(xq2):
                m = xq2[nx2]
                nc.gpsimd.dma_start(xhi_sb[m], xhi_d[m])
                nc.gpsimd.dma_start(xlo_sb[m], xlo_d[m])
                nx2 += 1

        # ---- phase 2: remaining m-blocks
        if spill_mode:
            for m in range(window, 2 * window):
                open_ten(m, KH, KK, 0, False)
                rbase.pop(m, None)
                for s in range(KH, KK):
                    legs_hi(m, s)
                    legs_lo(m, s)
                finalize(m)
            rest = range(2 * window, MB)
        else:
            rest = range(window, MB)
        for m in rest:
            open_ten(m, 0, KK, 0, True)
            rbase.pop(m, None)
            for s in range(KK):
                legs_hi(m, s)
                legs_lo(m, s)
            finalize(m)

    _legalize_waits(nc)
    return nc


def _stage_inputs(input, binary, alpha, bias, xlo_dt="e4"):
    np_e4 = mybir.dt.np(E4)
    np_xlo = mybir.dt.np({"e5": E5, "e4": E4}[xlo_dt])

    x = np.ascontiguousarray(np.asarray(input, dtype=np.float32)).reshape(MS, I)
    x_hi = x.astype(np_e4)
    x_lo = (x - x_hi.astype(np.float32)).astype(np_xlo)
    # [MS, I] -> [m, j, kk, t, p] -> [m, p, kk, t, j]
    def relayout_x(a):
        return np.ascontiguousarray(
            a.reshape(MB, P, KK, 2, P).transpose(0, 4, 2, 3, 1))
    xhi = relayout_x(x_hi)
    xlo = relayout_x(x_lo)

    binary = np.asarray(binary, dtype=np.float32)
    alpha = np.ascontiguousarray(np.asarray(alpha, dtype=np.float32))
    bias = np.asarray(bias, dtype=np.float32)

    in_maps = []
    for c in range(N_CORES):
        sl = slice(c * O_SH, (c + 1) * O_SH)
        # binary [512, G, A, NB] -> [ot, p, kk, t, a, b] -> [kk, p, ot, b, t, a]
        bc = binary[sl].reshape(OT, P, KK, 2, A, NB)
        bq = np.ascontiguousarray(bc.transpose(2, 1, 0, 5, 3, 4)).astype(np_e4)
        al = np.ascontiguousarray(
            alpha[sl].reshape(OT, P, G, NB).transpose(1, 0, 2, 3))
        in_maps.append({
            "xhi": xhi,
            "xlo": xlo,
            "bq": bq,
            "al": al,
            "biasr": np.ascontiguousarray(
                np.broadcast_to(bias[sl][None, :], (P, O_SH))),
        })
    return in_maps


def kernel(input, binary, alpha, bias, _trace=False, **cfg):
    key = tuple(sorted(cfg.items()))
    if key not in _CACHED:
        _CACHED[key] = build_nc(**cfg)
    nc = _CACHED[key]
    in_maps = _stage_inputs(input, binary, alpha, bias,
                            xlo_dt=cfg.get("xlo_dt", "e4"))
    res = run_bass_kernel_spmd(nc, in_maps, core_ids=list(range(N_CORES)),
                               trace=_trace)
    out = np.concatenate(
        [np.asarray(res.results[c]["out"]) for c in range(N_CORES)],
        axis=1).astype(np.float32).reshape(B, S, O)
    kernel.last_result = res
    return out
